# revision 5
# baseline (speedup 1.0000x reference)
"""VQ codebook EMA kernel for 8 Trainium2 NeuronCores.

Data-parallel: x [64,256,32,32] sharded over batch (8 b-blocks/core);
codebook [256,1024] replicated; per-core cluster counts + centroid sums
all-reduced on device before the EMA normalize and gather.

v2: fp32 dist matmuls kept (exact argmin) but the -||c||^2 bias row is a
2-deep fp16 split matmul (hi+lo, exact to ~1e-4) instead of 4 fp32
passes; S is copied out of PSUM by the scalar engine so the next chunk's
dist matmuls don't wait on max/onehot; output is gathered from an SBUF
fp32 table via gpsimd ap_gather (no DRAM round-trip, no PE transposes).
"""
import sys
sys.path.insert(0, "/opt/pypackages")
sys.path.insert(0, "/opt/trn_rl_repo")
import numpy as np
import concourse.bass as bass
import concourse.mybir as mybir
import concourse.tile as tile
from concourse import bacc
from concourse.bass_utils import run_bass_kernel_spmd

N_CORES = 8
B, C, H, W = 64, 256, 32, 32
F, K = 256, 1024
B_LOC = B // N_CORES           # 8 b-blocks per core
HW = H * W                     # 1024 tokens per b-block
N_CHUNK = B_LOC * (HW // 128)  # 64 chunks of 128 tokens
N_TOK = N_CHUNK * 128          # 8192 tokens per core
BIG = 16384.0                  # 2^14: exact scaling; +1 survives ulp(BIG*m)
DECAY = 0.99
EPS = 1e-05

f32 = mybir.dt.float32
f16 = mybir.dt.float16
i16 = mybir.dt.int16
u32 = mybir.dt.uint32

_NC = None


def _build():
    nc = bacc.Bacc("TRN2", target_bir_lowering=False, debug=False,
                   num_devices=N_CORES)
    x_d = nc.dram_tensor("x", [B_LOC, C, H, W], f32, kind="ExternalInput").ap()
    cent_d = nc.dram_tensor("centroids", [C, K], f32, kind="ExternalInput").ap()
    cs_d = nc.dram_tensor("cluster_size", [K], f32, kind="ExternalInput").ap()
    avg_d = nc.dram_tensor("centroids_avg", [C, K], f32, kind="ExternalInput").ap()
    out_d = nc.dram_tensor("out", [B_LOC, C, H, W], f32, kind="ExternalOutput").ap()

    x_v = x_d.rearrange("b (i p) h w -> b i p (h w)", p=128)     # [8, 2, 128, 1024]
    cent_v = cent_d.rearrange("(i p) k -> i p k", p=128)          # [2, 128, 1024]
    avg_v = avg_d.rearrange("(i p) k -> i p k", p=128)
    cs_v = cs_d.rearrange("(s k) -> s k", s=1)                    # [1, 1024]
    out_v = out_d.rearrange("b (i p) h w -> b i p (h w)", p=128)

    mul = mybir.AluOpType.mult
    add = mybir.AluOpType.add
    sub = mybir.AluOpType.subtract

    with tile.TileContext(nc, num_cores=N_CORES) as tc:
        with (
            tc.tile_pool(name="const", bufs=1) as constp,
            tc.tile_pool(name="xpool", bufs=2) as xpool,
            tc.tile_pool(name="work", bufs=1) as work,
            tc.tile_pool(name="small", bufs=2) as small,
            tc.tile_pool(name="dram", bufs=1, space="DRAM") as dram,
        ):
            # ---------------- constants / setup ----------------
            ones2 = constp.tile([2, 128], f16)       # bias stationary
            nc.vector.memset(ones2[:], 1.0)
            ones_col32 = constp.tile([128, 1], f32)  # for ||c||^2 column sums
            nc.vector.memset(ones_col32[:], 1.0)
            ones_row32 = constp.tile([1, 128], f32)  # for M partition-bcast
            nc.vector.memset(ones_row32[:], 1.0)
            ones_col16 = constp.tile([128, 1], f16)  # cnt stationary
            nc.vector.memset(ones_col16[:], 1.0)

            cents2 = [constp.tile([128, K], f32, name=f"cent2{i}") for i in range(2)]
            cent_sb = [constp.tile([128, K], f32, name=f"cent{i}") for i in range(2)]
            for i in range(2):
                nc.sync.dma_start(cent_sb[i][:], cent_v[i])
                nc.vector.tensor_scalar_mul(cents2[i][:], cent_sb[i][:], 2.0)

            cs_row = constp.tile([1, K], f32)
            nc.sync.dma_start(cs_row[:], cs_v)
            avgs = [constp.tile([128, K], f32, name=f"avg{i}") for i in range(2)]
            for i in range(2):
                nc.sync.dma_start(avgs[i][:], avg_v[i])

            ind_all8 = constp.tile([128, N_CHUNK, 8], u32)  # FI8 writes 8/chunk

            ccin = dram.tile([257, K], f32)
            ccout = dram.tile([257, K], f32, addr_space="Shared")

            with tc.tile_pool(name="psum1", bufs=1, space="PSUM") as psum1:
                # ||c||^2 -> 2-row fp16 split bias (uses the S slot pre-loop)
                c2ps = psum1.tile([1, K], f32, tag="S", name="c2ps")
                sq = work.tile([128, K], f32, tag="sq")
                for i in range(2):
                    nc.vector.tensor_tensor(out=sq[:], in0=cent_sb[i][:],
                                            in1=cent_sb[i][:], op=mul)
                    for h in range(2):
                        nc.tensor.matmul(c2ps[:, h*512:(h+1)*512], ones_col32[:],
                                         sq[:, h*512:(h+1)*512],
                                         start=(i == 0), stop=(i == 1))
                negc2 = constp.tile([1, K], f32)
                nc.vector.tensor_scalar_mul(negc2[:], c2ps[:], -1.0)
                nc2 = constp.tile([2, K], f16)        # [hi, lo] split of -c^2
                nc.vector.tensor_copy(nc2[0:1, :], negc2[:])
                c2h32 = small.tile([1, K], f32, tag="c2h32")
                nc.vector.tensor_copy(c2h32[:], nc2[0:1, :])
                resid = small.tile([1, K], f32, tag="resid")
                nc.vector.tensor_tensor(out=resid[:], in0=negc2[:],
                                        in1=c2h32[:], op=sub)
                resid16 = small.tile([1, K], f16, tag="resid16")
                nc.vector.tensor_copy(resid16[:], resid[:])
                # DVE can't write at partition offset 1; DMA can
                nc.sync.dma_start(nc2[1:2, :], resid16[:])

                segps = [psum1.tile([128, K], f32, name=f"segp{i}") for i in range(2)]
                cntps = psum1.tile([1, K], f32, tag="cntps", name="cntps")

                # ---------------- phase 1: 64 chunks ----------------
                for bi in range(B_LOC):
                    xts = [xpool.tile([128, HW], f32, name=f"xt{i}", tag=f"xt{i}")
                           for i in range(2)]
                    xt16s = [xpool.tile([128, HW], f16, name=f"xt16{i}",
                                        tag=f"xt16{i}") for i in range(2)]
                    xf16 = xpool.tile([128, 16 * 128], f16, tag="xf16")
                    for i in range(2):
                        nc.sync.dma_start(xts[i][:], x_v[bi, i])
                        nc.scalar.copy(xt16s[i][:], xts[i][:])
                    for t in range(8):
                        for i in range(2):
                            nc.sync.dma_start_transpose(
                                xf16[:, (t*2+i)*128:(t*2+i+1)*128],
                                xt16s[i][:, t*128:(t+1)*128])

                    for t in range(8):
                        ci = bi * 8 + t
                        S = psum1.tile([128, K], f32, tag="S", name=f"S_{ci}")
                        for h in range(2):
                            hs = slice(h*512, (h+1)*512)
                            nc.tensor.matmul(S[:, hs], ones2[:], nc2[:, hs],
                                             start=True, stop=False,
                                             skip_group_check=True)
                            for i in range(2):
                                nc.tensor.matmul(S[:, hs],
                                                 xts[i][:, t*128:(t+1)*128],
                                                 cents2[i][:, hs],
                                                 start=False, stop=(i == 1),
                                                 skip_group_check=True)

                        # scalar engine drains S from PSUM so the next
                        # chunk's matmuls don't wait on max/onehot readers
                        S_sb = work.tile([128, K], f32, tag="S_sb", bufs=2)
                        nc.scalar.copy(S_sb[:], S[:])

                        m8 = small.tile([128, 8], f32, tag="m8")
                        nc.vector.max(out=m8[:], in_=S_sb[:])
                        bias = small.tile([128, 1], f32, tag="bias")
                        nc.vector.tensor_scalar(out=bias[:], in0=m8[:, 0:1],
                                                scalar1=-BIG, scalar2=1.0,
                                                op0=mul, op1=add)
                        onehot = work.tile([128, K], f16, tag="onehot", bufs=3)
                        nc.scalar.activation(onehot[:], S_sb[:],
                                             mybir.ActivationFunctionType.Relu,
                                             bias=bias[:], scale=BIG)
                        nc.vector.max_index(out=ind_all8[:, ci, :],
                                            in_max=m8[:], in_values=S_sb[:])

                        for i in range(2):
                            for h in range(2):
                                nc.tensor.matmul(
                                    segps[i][:, h*512:(h+1)*512],
                                    xf16[:, (t*2+i)*128:(t*2+i+1)*128],
                                    onehot[:, h*512:(h+1)*512],
                                    start=(ci == 0), stop=(ci == N_CHUNK - 1),
                                    skip_group_check=True)
                        for h in range(2):
                            nc.tensor.matmul(cntps[:, h*512:(h+1)*512],
                                             ones_col16[:],
                                             onehot[:, h*512:(h+1)*512],
                                             start=(ci == 0),
                                             stop=(ci == N_CHUNK - 1),
                                             skip_group_check=True)

                # ------- flush partials (scaled by 1-decay) -------
                for i in range(2):
                    fl = work.tile([128, K], f32, name=f"fl{i}", tag="flush",
                                   bufs=2)
                    nc.vector.tensor_scalar_mul(fl[:], segps[i][:], 1.0 - DECAY)
                    nc.sync.dma_start(ccin[i*128:(i+1)*128, :], fl[:])
                cfl = work.tile([1, K], f32, tag="cflush")
                nc.vector.tensor_scalar_mul(cfl[:], cntps[:], 1.0 - DECAY)
                nc.sync.dma_start(ccin[256:257, :], cfl[:])

            # psum1 released; allreduce overlaps the gather-index build
            nc.gpsimd.collective_compute(
                "AllReduce", mybir.AluOpType.add,
                replica_groups=[list(range(N_CORES))],
                ins=[ccin.opt()], outs=[ccout.opt()],
            )

            # ---- wrapped gather indices (independent of the collective) ----
            # ap_gather unwraps idx[p, s] -> token j = s*16 + p per 16-part
            # group; token j of b-block bi at (t, r=u*16+p) sits at free slot
            # s = ci*8 + u.  8 strided DMAs + 3 doubling replications.
            ind16_64 = constp.tile([128, N_CHUNK], i16)
            nc.vector.tensor_copy(ind16_64[:], ind_all8[:, :, 0])
            wv = constp.tile([128, N_CHUNK, 8], i16)
            for u in range(8):
                nc.sync.dma_start(wv[0:16, :, u], ind16_64[u*16:(u+1)*16, :])
            nc.sync.dma_start(wv[16:32], wv[0:16])
            nc.sync.dma_start(wv[32:64], wv[0:32])
            nc.sync.dma_start(wv[64:128], wv[0:64])

            # ---- EMA + normalize (all on natural [*, K] layout) ----
            seg_g = [work.tile([128, K], f32, name=f"segg{i}", tag=f"segg{i}")
                     for i in range(2)]
            for i in range(2):
                nc.sync.dma_start(seg_g[i][:], ccout[i*128:(i+1)*128, :])
            cnt_row = small.tile([1, K], f32, tag="cnt_row")
            nc.sync.dma_start(cnt_row[:], ccout[256:257, :])

            new_cs = small.tile([1, K], f32, tag="new_cs")
            nc.vector.scalar_tensor_tensor(out=new_cs[:], in0=cs_row[:],
                                           scalar=DECAY, in1=cnt_row[:],
                                           op0=mul, op1=add)
            n_sc = small.tile([1, 1], f32, tag="n_sc")
            nc.vector.reduce_sum(n_sc[:], new_cs[:], axis=mybir.AxisListType.X)
            denom = small.tile([1, 1], f32, tag="denom")
            nc.vector.tensor_scalar_add(denom[:], n_sc[:], float(K) * EPS)
            rcp_n = small.tile([1, 1], f32, tag="rcp_n")
            nc.vector.reciprocal(rcp_n[:], n_sc[:])
            fmul = small.tile([1, 1], f32, tag="fmul")
            nc.vector.tensor_mul(fmul[:], denom[:], rcp_n[:])
            t1 = small.tile([1, K], f32, tag="t1")
            nc.vector.tensor_scalar_add(t1[:], new_cs[:], EPS)
            r1 = small.tile([1, K], f32, tag="r1")
            nc.vector.reciprocal(r1[:], t1[:])
            M_row = small.tile([1, K], f32, tag="M_row")
            nc.vector.tensor_scalar_mul(M_row[:], r1[:], fmul[:])
            # broadcast M to all partitions on the (idle) PE: ones^T @ M_row
            with tc.tile_pool(name="psum2", bufs=1, space="PSUM") as psum2:
                Mrep = psum2.tile([128, K], f32, tag="Mrep")
                for h in range(2):
                    nc.tensor.matmul(Mrep[:, h*512:(h+1)*512], ones_row32[:],
                                     M_row[:, h*512:(h+1)*512],
                                     start=True, stop=True)

                # table[i][f, k] = (decay*avg + seg_g) * M  (fp32, in SBUF)
                table = [work.tile([128, K], f32, name=f"table{i}",
                                   tag=f"tab{i}") for i in range(2)]
                for i in range(2):
                    nc.vector.scalar_tensor_tensor(out=table[i][:],
                                                   in0=avgs[i][:],
                                                   scalar=DECAY,
                                                   in1=seg_g[i][:],
                                                   op0=mul, op1=add)
                    nc.vector.tensor_tensor(out=table[i][:], in0=table[i][:],
                                            in1=Mrep[:], op=mul)

            # ---- phase 2: SBUF gather per b-block + direct DMA out ----
            for bi in range(B_LOC):
                idx_slice = wv[:, bi*8:(bi+1)*8, :].rearrange("p a b -> p (a b)")
                for i in range(2):
                    conv = work.tile([128, HW], f32, name=f"conv{i}",
                                     tag=f"conv{i}", bufs=2)
                    nc.gpsimd.ap_gather(
                        conv[:].rearrange("p (k d) -> p k d", d=1),
                        table[i][:].rearrange("p (k d) -> p k d", d=1),
                        idx_slice,
                        channels=128, num_elems=K, d=1, num_idxs=HW)
                    nc.sync.dma_start(out_v[bi, i], conv[:])

    nc.finalize()
    return nc


def _get_nc():
    global _NC
    if _NC is None:
        _NC = _build()
    return _NC


def kernel(x, centroids, cluster_size, centroids_avg):
    x = np.ascontiguousarray(np.asarray(x, dtype=np.float32))
    centroids = np.ascontiguousarray(np.asarray(centroids, dtype=np.float32))
    cluster_size = np.ascontiguousarray(np.asarray(cluster_size, dtype=np.float32))
    centroids_avg = np.ascontiguousarray(np.asarray(centroids_avg, dtype=np.float32))
    nc = _get_nc()
    in_maps = []
    for i in range(N_CORES):
        in_maps.append({
            "x": x[i*B_LOC:(i+1)*B_LOC],
            "centroids": centroids,
            "cluster_size": cluster_size,
            "centroids_avg": centroids_avg,
        })
    res = run_bass_kernel_spmd(nc, in_maps, core_ids=list(range(N_CORES)))
    out = np.concatenate([res.results[i]["out"] for i in range(N_CORES)], axis=0)
    return out


if __name__ == "__main__":
    rng = np.random.default_rng(0)
    xs = rng.standard_normal((B, C, H, W), dtype=np.float32)
    cs = rng.standard_normal((C, K), dtype=np.float32)
    sz = rng.random(K, dtype=np.float32)
    av = rng.standard_normal((C, K), dtype=np.float32)
    out = kernel(xs, cs, sz, av)
    print("out", out.shape, out.dtype)


# revision 6
# speedup vs baseline: 1.9568x; 1.9568x over previous
"""VQ codebook EMA kernel for 8 Trainium2 NeuronCores.

Data-parallel: x [64,256,32,32] sharded over batch (8 b-blocks/core);
codebook [256,1024] replicated; per-core cluster counts + centroid sums
all-reduced on device before the EMA normalize and gather.

v3: dist matmuls in fp16 via an exact-enough 3-term split
(x_h*c_h + x_h*c_l + x_l*c_h, fp32-exact ||c||^2 bias as a 2-deep fp16
split row) — verified 0 argmin flips vs fp32 on the reference inputs.
This replaces fp32 LOW_HIGH double-pass matmuls (~2.4x slower each).
S is drained from PSUM by the scalar engine so the next chunk's dist
matmuls don't serialize behind max/onehot. Output via the SWDGE DRAM
gather spread over 4 SWDGE queues.
"""
import sys
sys.path.insert(0, "/opt/pypackages")
sys.path.insert(0, "/opt/trn_rl_repo")
import numpy as np
import concourse.bass as bass
import concourse.mybir as mybir
import concourse.tile as tile
from concourse import bacc, bass_isa
from concourse.bass_utils import run_bass_kernel_spmd
from concourse.masks import make_identity

N_CORES = 8
B, C, H, W = 64, 256, 32, 32
F, K = 256, 1024
B_LOC = B // N_CORES           # 8 b-blocks per core
HW = H * W                     # 1024 tokens per b-block
N_CHUNK = B_LOC * (HW // 128)  # 64 chunks of 128 tokens
N_TOK = N_CHUNK * 128          # 8192 tokens per core
BIG = 16384.0                  # 2^14: exact scaling; +1 survives ulp(BIG*m)
DECAY = 0.99
EPS = 1e-05

f32 = mybir.dt.float32
f16 = mybir.dt.float16
i16 = mybir.dt.int16
u32 = mybir.dt.uint32

_NC = None


def _build():
    nc = bacc.Bacc("TRN2", target_bir_lowering=False, debug=False,
                   num_devices=N_CORES, num_swdge_queues=4)
    x_d = nc.dram_tensor("x", [B_LOC, C, H, W], f32, kind="ExternalInput").ap()
    cent_d = nc.dram_tensor("centroids", [C, K], f32, kind="ExternalInput").ap()
    cs_d = nc.dram_tensor("cluster_size", [K], f32, kind="ExternalInput").ap()
    avg_d = nc.dram_tensor("centroids_avg", [C, K], f32, kind="ExternalInput").ap()
    out_d = nc.dram_tensor("out", [B_LOC, C, H, W], f32, kind="ExternalOutput").ap()

    x_v = x_d.rearrange("b (i p) h w -> b i p (h w)", p=128)     # [8, 2, 128, 1024]
    cent_v = cent_d.rearrange("(i p) k -> i p k", p=128)          # [2, 128, 1024]
    avg_v = avg_d.rearrange("(i p) k -> i p k", p=128)
    cs8_v = cs_d.rearrange("(s p) -> s p", p=128)                 # [8, 128]
    out_v = out_d.rearrange("b (i p) h w -> b i p (h w)", p=128)

    mul = mybir.AluOpType.mult
    add = mybir.AluOpType.add
    sub = mybir.AluOpType.subtract

    with tile.TileContext(nc, num_cores=N_CORES) as tc:
        with (
            tc.tile_pool(name="const", bufs=1) as constp,
            tc.tile_pool(name="xpool", bufs=2) as xpool,
            tc.tile_pool(name="work", bufs=1) as work,
            tc.tile_pool(name="small", bufs=2) as small,
            tc.tile_pool(name="dram", bufs=1, space="DRAM") as dram,
        ):
            # ---------------- constants / setup ----------------
            ident = constp.tile([128, 128], f32)
            make_identity(nc, ident[:])
            ones2 = constp.tile([2, 128], f16)       # bias stationary
            nc.vector.memset(ones2[:], 1.0)
            ones_col32 = constp.tile([128, 1], f32)  # for ||c||^2 column sums
            nc.vector.memset(ones_col32[:], 1.0)
            ones_col16 = constp.tile([128, 1], f16)  # cnt stationary
            nc.vector.memset(ones_col16[:], 1.0)

            # fp16 split of 2*centroids: ch2 + cl2 ~= 2c to ~2^-22
            cent_sb = [constp.tile([128, K], f32, name=f"cent{i}") for i in range(2)]
            ch2 = [constp.tile([128, K], f16, name=f"ch2_{i}") for i in range(2)]
            cl2 = [constp.tile([128, K], f16, name=f"cl2_{i}") for i in range(2)]
            c2t = work.tile([128, K], f32, tag="c2t")
            for i in range(2):
                nc.sync.dma_start(cent_sb[i][:], cent_v[i])
                nc.vector.tensor_scalar_mul(c2t[:], cent_sb[i][:], 2.0)
                nc.vector.tensor_copy(ch2[i][:], c2t[:])
                # cl2 = (2c * 1.0) - ch2   (mixed-dtype STT, out fp16)
                nc.vector.scalar_tensor_tensor(out=cl2[i][:], in0=c2t[:],
                                               scalar=1.0, in1=ch2[i][:],
                                               op0=mul, op1=sub)

            cs8 = constp.tile([8, 128], f32)       # cluster_size as [s, p]
            nc.sync.dma_start(cs8[:], cs8_v)
            avgs = [constp.tile([128, K], f32, name=f"avg{i}") for i in range(2)]
            for i in range(2):
                nc.sync.dma_start(avgs[i][:], avg_v[i])

            ind_all8 = constp.tile([128, N_CHUNK, 8], u32)  # FI8 writes 8/chunk

            ccin = dram.tile([257, K], f32)
            ccout = dram.tile([257, K], f32, addr_space="Shared")
            tab16_dram = dram.tile([K, F], f16)

            with tc.tile_pool(name="psum1", bufs=1, space="PSUM") as psum1:
                # ||c||^2 -> 2-row fp16 split bias (uses the S slot pre-loop)
                c2ps = psum1.tile([1, K], f32, tag="S", name="c2ps")
                sq = work.tile([128, K], f32, tag="sq")
                for i in range(2):
                    nc.vector.tensor_tensor(out=sq[:], in0=cent_sb[i][:],
                                            in1=cent_sb[i][:], op=mul)
                    for h in range(2):
                        nc.tensor.matmul(c2ps[:, h*512:(h+1)*512], ones_col32[:],
                                         sq[:, h*512:(h+1)*512],
                                         start=(i == 0), stop=(i == 1))
                negc2 = constp.tile([1, K], f32)
                nc.vector.tensor_scalar_mul(negc2[:], c2ps[:], -1.0)
                nc2 = constp.tile([2, K], f16)        # [hi, lo] split of -c^2
                nc.vector.tensor_copy(nc2[0:1, :], negc2[:])
                resid16 = small.tile([1, K], f16, tag="resid16")
                nc.vector.scalar_tensor_tensor(out=resid16[:], in0=negc2[:],
                                               scalar=1.0, in1=nc2[0:1, :],
                                               op0=mul, op1=sub)
                # DVE can't write at partition offset 1; DMA can
                nc.sync.dma_start(nc2[1:2, :], resid16[:])

                segps = [psum1.tile([128, K], f32, name=f"segp{i}") for i in range(2)]
                cntps = psum1.tile([1, K], f32, tag="cntps", name="cntps")

                # ---------------- phase 1: 64 chunks ----------------
                for bi in range(B_LOC):
                    xts = [xpool.tile([128, HW], f32, name=f"xt{i}", tag=f"xt{i}")
                           for i in range(2)]
                    xhs = [xpool.tile([128, HW], f16, name=f"xh{i}",
                                      tag=f"xh{i}") for i in range(2)]
                    xls = [xpool.tile([128, HW], f16, name=f"xl{i}",
                                      tag=f"xl{i}") for i in range(2)]
                    xf16 = xpool.tile([128, 16 * 128], f16, tag="xf16")
                    for i in range(2):
                        nc.sync.dma_start(xts[i][:], x_v[bi, i])
                        nc.scalar.copy(xhs[i][:], xts[i][:])
                        # x_l = x - fp16(x), rounded to fp16 (exact-ish)
                        nc.vector.scalar_tensor_tensor(out=xls[i][:],
                                                       in0=xts[i][:],
                                                       scalar=1.0,
                                                       in1=xhs[i][:],
                                                       op0=mul, op1=sub)
                    for t in range(8):
                        for i in range(2):
                            nc.sync.dma_start_transpose(
                                xf16[:, (t*2+i)*128:(t*2+i+1)*128],
                                xhs[i][:, t*128:(t+1)*128])

                    for t in range(8):
                        ci = bi * 8 + t
                        tok = slice(t*128, (t+1)*128)
                        S = psum1.tile([128, K], f32, tag="S", name=f"S_{ci}")
                        for h in range(2):
                            hs = slice(h*512, (h+1)*512)
                            nc.tensor.matmul(S[:, hs], ones2[:], nc2[:, hs],
                                             start=True, stop=False,
                                             skip_group_check=True)
                        for i in range(2):
                            for xop, cop, last in ((xhs[i], ch2[i], False),
                                                   (xhs[i], cl2[i], False),
                                                   (xls[i], ch2[i], i == 1)):
                                for h in range(2):
                                    hs = slice(h*512, (h+1)*512)
                                    nc.tensor.matmul(S[:, hs], xop[:, tok],
                                                     cop[:, hs], start=False,
                                                     stop=(last and h == 1),
                                                     skip_group_check=True)

                        # scalar engine drains S from PSUM so the next
                        # chunk's matmuls don't wait on max/onehot readers
                        S_sb = work.tile([128, K], f32, tag="S_sb", bufs=2)
                        nc.scalar.copy(S_sb[:], S[:])

                        m8 = small.tile([128, 8], f32, tag="m8")
                        nc.vector.max(out=m8[:], in_=S_sb[:])
                        bias = small.tile([128, 1], f32, tag="bias")
                        nc.vector.tensor_scalar(out=bias[:], in0=m8[:, 0:1],
                                                scalar1=-BIG, scalar2=1.0,
                                                op0=mul, op1=add)
                        onehot = work.tile([128, K], f16, tag="onehot", bufs=3)
                        nc.scalar.activation(onehot[:], S_sb[:],
                                             mybir.ActivationFunctionType.Relu,
                                             bias=bias[:], scale=BIG)
                        nc.vector.max_index(out=ind_all8[:, ci, :],
                                            in_max=m8[:], in_values=S_sb[:])

                        for i in range(2):
                            for h in range(2):
                                nc.tensor.matmul(
                                    segps[i][:, h*512:(h+1)*512],
                                    xf16[:, (t*2+i)*128:(t*2+i+1)*128],
                                    onehot[:, h*512:(h+1)*512],
                                    start=(ci == 0), stop=(ci == N_CHUNK - 1),
                                    skip_group_check=True)
                        for h in range(2):
                            nc.tensor.matmul(cntps[:, h*512:(h+1)*512],
                                             ones_col16[:],
                                             onehot[:, h*512:(h+1)*512],
                                             start=(ci == 0),
                                             stop=(ci == N_CHUNK - 1),
                                             skip_group_check=True)

                # ------- flush partials (scaled by 1-decay) -------
                for i in range(2):
                    fl = work.tile([128, K], f32, name=f"fl{i}", tag="flush",
                                   bufs=2)
                    nc.vector.tensor_scalar_mul(fl[:], segps[i][:], 1.0 - DECAY)
                    nc.sync.dma_start(ccin[i*128:(i+1)*128, :], fl[:])
                cfl = work.tile([1, K], f32, tag="cflush")
                nc.vector.tensor_scalar_mul(cfl[:], cntps[:], 1.0 - DECAY)
                nc.sync.dma_start(ccin[256:257, :], cfl[:])

            # psum1 released; allreduce overlaps the wrapped-idx build
            nc.gpsimd.collective_compute(
                "AllReduce", mybir.AluOpType.add,
                replica_groups=[list(range(N_CORES))],
                ins=[ccin.opt()], outs=[ccout.opt()],
            )

            # ---- wrapped gather indices (independent of the collective) ----
            # gather slot (r, j) with j = u*64 + c holds ind of token
            # t = 128*c + 16*u + r  -> 8 contiguous [16, 64] copies.
            ind16 = constp.tile([128, N_CHUNK], i16)
            nc.vector.tensor_copy(ind16[:], ind_all8[:, :, 0])
            wrapped = constp.tile([128, N_TOK // 16], i16)
            for u in range(8):
                nc.sync.dma_start(wrapped[0:16, 64*u:64*(u+1)],
                                  ind16[16*u:16*(u+1), :])
            for g in range(1, 8):
                nc.sync.dma_start(wrapped[g*16:(g+1)*16, :], wrapped[0:16, :])

            with tc.tile_pool(name="psum2", bufs=2, space="PSUM") as psum2:
                # ---- EMA + normalize ----
                seg_g = [work.tile([128, K], f32, name=f"segg{i}", tag=f"segg{i}")
                         for i in range(2)]
                for i in range(2):
                    nc.sync.dma_start(seg_g[i][:], ccout[i*128:(i+1)*128, :])
                cnt8 = small.tile([8, 128], f32, tag="cnt8")
                nc.sync.dma_start(cnt8[:], ccout[256:257, :].rearrange(
                    "one (s p) -> (one s) p", p=128))
                cntT_ps = psum2.tile([128, 8], f32, tag="cntT_ps", bufs=1)
                nc.tensor.transpose(cntT_ps[:], cnt8[:], ident[0:8, 0:8])
                cntT = small.tile([128, 8], f32, tag="cntT")
                nc.vector.tensor_copy(cntT[:], cntT_ps[:])
                cs8T_ps = psum2.tile([128, 8], f32, tag="cs8T_ps", bufs=1)
                nc.tensor.transpose(cs8T_ps[:], cs8[:], ident[0:8, 0:8])

                new_csT = small.tile([128, 8], f32, tag="new_csT")
                nc.vector.tensor_scalar_mul(new_csT[:], cs8T_ps[:], DECAY)
                nc.vector.tensor_add(new_csT[:], new_csT[:], cntT[:])
                psum_n = small.tile([128, 1], f32, tag="psum_n")
                nc.vector.reduce_sum(psum_n[:], new_csT[:],
                                     axis=mybir.AxisListType.X)
                n_all = small.tile([128, 1], f32, tag="n_all")
                nc.gpsimd.partition_all_reduce(n_all[:], psum_n[:], channels=128,
                                               reduce_op=bass_isa.ReduceOp.add)
                # M[k] = 1/cs_norm[k] = (n + K*eps)/n * 1/(new_cs + eps)
                denom = small.tile([128, 1], f32, tag="denom")
                nc.vector.tensor_scalar_add(denom[:], n_all[:], float(K) * EPS)
                rcp_n = small.tile([128, 1], f32, tag="rcp_n")
                nc.vector.reciprocal(rcp_n[:], n_all[:])
                fmul = small.tile([128, 1], f32, tag="fmul")
                nc.vector.tensor_mul(fmul[:], denom[:], rcp_n[:])
                t1 = small.tile([128, 8], f32, tag="t1")
                nc.vector.tensor_scalar_add(t1[:], new_csT[:], EPS)
                r1 = small.tile([128, 8], f32, tag="r1")
                nc.vector.reciprocal(r1[:], t1[:])
                Mt = small.tile([128, 8], f32, tag="Mt")
                nc.vector.tensor_scalar_mul(Mt[:], r1[:], fmul[:])

                newavg = [work.tile([128, K], f32, name=f"newavg{i}",
                                    tag=f"nav{i}") for i in range(2)]
                for i in range(2):
                    nc.vector.scalar_tensor_tensor(out=newavg[i][:],
                                                   in0=avgs[i][:],
                                                   scalar=DECAY,
                                                   in1=seg_g[i][:],
                                                   op0=mul, op1=add)

                # ---- table: new_centroids^T [K, F] fp16 in DRAM ----
                tabv = tab16_dram.rearrange("(s p) f -> p s f", p=128)
                for s in range(8):
                    tab_sb = work.tile([128, F], f16, tag="tab_sb", bufs=2)
                    for hh in range(2):
                        tps = psum2.tile([128, 128], f32, tag="tps",
                                         name=f"tps{s}_{hh}")
                        nc.tensor.transpose(tps[:],
                                            newavg[hh][:, s*128:(s+1)*128],
                                            ident[:])
                        nc.vector.tensor_scalar_mul(tab_sb[:, hh*128:(hh+1)*128],
                                                    tps[:], Mt[:, s:s+1])
                    nc.sync.dma_start(tabv[:, s, :], tab_sb[:])

                # ---- phase 2: gathers spread over 4 SWDGE queues ----
                # 16 gather calls of 512 idxs (>=1024 per call overruns the
                # SWDGE descriptor fifo). call q=(u,ch); i_loc=(bl cc r).
                gath = work.tile([128, 16, 2, 512], f16, tag="gath")
                wv = wrapped.rearrange("p (q j) -> p q j", q=16)
                for q in range(16):
                    nc.gpsimd.dma_gather(gath[:, q], tab16_dram, wv[:, q],
                                         num_idxs=512, num_idxs_reg=512,
                                         elem_size=F, transpose=True,
                                         queue_num=q % 4)
                gv = gath.rearrange("p (u ch) g (bl cc r) -> p g ch bl cc u r",
                                    u=8, ch=2, bl=4, cc=8, r=16)
                for bi in range(B_LOC):
                    for i in range(2):
                        conv = work.tile([128, 8, 8, 16], f32, name=f"conv{i}",
                                         tag=f"conv{i}", bufs=2)
                        if i == 0:
                            nc.vector.tensor_copy(conv[:], gv[:, i, bi // 4, bi % 4])
                        else:
                            nc.scalar.copy(conv[:], gv[:, i, bi // 4, bi % 4])
                        nc.sync.dma_start(out_v[bi, i],
                                          conv.rearrange("p a b c -> p (a b c)"))

    nc.finalize()
    return nc


def _get_nc():
    global _NC
    if _NC is None:
        _NC = _build()
    return _NC


def kernel(x, centroids, cluster_size, centroids_avg):
    x = np.ascontiguousarray(np.asarray(x, dtype=np.float32))
    centroids = np.ascontiguousarray(np.asarray(centroids, dtype=np.float32))
    cluster_size = np.ascontiguousarray(np.asarray(cluster_size, dtype=np.float32))
    centroids_avg = np.ascontiguousarray(np.asarray(centroids_avg, dtype=np.float32))
    nc = _get_nc()
    in_maps = []
    for i in range(N_CORES):
        in_maps.append({
            "x": x[i*B_LOC:(i+1)*B_LOC],
            "centroids": centroids,
            "cluster_size": cluster_size,
            "centroids_avg": centroids_avg,
        })
    res = run_bass_kernel_spmd(nc, in_maps, core_ids=list(range(N_CORES)))
    out = np.concatenate([res.results[i]["out"] for i in range(N_CORES)], axis=0)
    return out


if __name__ == "__main__":
    rng = np.random.default_rng(0)
    xs = rng.standard_normal((B, C, H, W), dtype=np.float32)
    cs = rng.standard_normal((C, K), dtype=np.float32)
    sz = rng.random(K, dtype=np.float32)
    av = rng.standard_normal((C, K), dtype=np.float32)
    out = kernel(xs, cs, sz, av)
    print("out", out.shape, out.dtype)
